# revision 17
# baseline (speedup 1.0000x reference)
"""Cross-attention Trainium2 kernel (nn_CrossAttention_48842368090446).

Problem: B=2, n=1024, N=4096, C=768, H=12, D=64.
  q = (q_in @ Wq)  -> [B,H,n,D]
  k,v = (x @ Wkv)  -> [B,H,N,D] each
  xattn = softmax(q k^T / sqrt(D))    (returned, [B,H,n,N])
  out = (xattn @ v reshaped) @ Wproj + bproj    (returned, [B,n,C])

Sharding: 8 cores = 2 batches x 4 head-groups (3 heads each).
Each core gets its batch's q_inT/xT (host-transposed layout prep, cast
to bf16) plus its heads' weight slices, and produces xattn_part
[3,1024,4096] (fp32) and an out_part [1024,768] fp32 partial (summed
over the 4 cores of a batch on the host = the Wproj-row-sharded
all-reduce; bias counted once via the leader core's bias input).

On-core dataflow (per head, pass A/B interleaved to keep the PE warm):
  - qT=[D,n] and kvT=[kT|vT] packed [128,N] via bf16 projections (k on
    partitions 0:64, v on 64:128); v natural via PE transposes of vT.
  - pass A: scores [n,N] tiles on PE (bf16 in, fp32 PSUM) -> ACT
    exp(scale=1/8) with accum_out row sums -> DVE reciprocal +
    tensor_scalar normalize in place (fp32) -> DMA xattn.
  - pass B: scoresT [N,n] tiles -> ACT exp (bf16 out) -> PE
    attn@[v|ones] accumulated into oT[65,n] fp32 (row 64 = Z) ->
    gpsimd-broadcast 1/Z -> oT_norm (f32r).
  - proj (per head, f32r): partial = oT_norm_h^T @ Wproj_h (+ bias on
    head 0), accumulated into SBUF via DVE adds.
"""
import numpy as np
import ml_dtypes

import concourse.bacc as bacc
import concourse.mybir as mybir
import concourse.tile as tile
from concourse import bass_utils
from concourse.masks import make_identity

F32 = mybir.dt.float32
F32R = mybir.dt.float32r
BF16 = mybir.dt.bfloat16
MM_DT = F32R     # output projection: ~1.6e-4 rounding
AT_DT = BF16     # attention + q/kv projections

B, N_Q, N_KV, C, H, D = 2, 1024, 4096, 768, 12, 64
HPC = 3            # heads per core
CT = C // 128      # 6 contraction tiles
EXP_FUNC = mybir.ActivationFunctionType.Exp
SCALE = float(D) ** -0.5


def build(mm_dt=MM_DT):
    nc = bacc.Bacc("TRN2")

    q_inT = nc.dram_tensor("q_inT", [C, N_Q], AT_DT, kind="ExternalInput").ap()
    xT = nc.dram_tensor("xT", [C, N_KV], AT_DT, kind="ExternalInput").ap()
    wq = nc.dram_tensor("wq", [C, HPC, 2 * D], AT_DT, kind="ExternalInput").ap()
    wkk = nc.dram_tensor("wkk", [C, HPC, 2 * D], AT_DT, kind="ExternalInput").ap()
    wv = nc.dram_tensor("wv", [C, 4 * D], AT_DT, kind="ExternalInput").ap()
    wproj = nc.dram_tensor("wproj", [HPC * D, C], mm_dt, kind="ExternalInput").ap()
    bias = nc.dram_tensor("bias", [1, C], mm_dt, kind="ExternalInput").ap()
    xattn = nc.dram_tensor("xattn_part", [HPC, N_Q, N_KV], F32, kind="ExternalOutput").ap()
    outp = nc.dram_tensor("out_part", [N_Q, C], F32, kind="ExternalOutput").ap()

    with tile.TileContext(nc) as tc:
        with tc.tile_pool(name="per", bufs=1) as per:
            # --- constants / weights (persistent) ---
            ident_f = per.tile([128, 128], F32, tag="ident_f")
            make_identity(nc, ident_f)
            identb = per.tile([128, 128], AT_DT, tag="identb")
            nc.vector.tensor_copy(identb, ident_f)

            ones_f = per.tile([128, 32], F32, tag="ones_f")
            nc.vector.memset(ones_f, 1.0)
            ones1_f = per.tile([1, 128], F32, tag="ones1_f")
            nc.vector.memset(ones1_f, 1.0)
            ones1 = per.tile([1, 128], mm_dt, tag="ones1")
            nc.vector.tensor_copy(ones1, ones1_f)

            wq_sb = per.tile([128, CT, HPC, 2 * D], AT_DT, tag="wq")
            nc.sync.dma_start(out=wq_sb, in_=wq.rearrange("(c p) h m -> p c h m", p=128))
            wkk_sb = per.tile([128, CT, HPC, 2 * D], AT_DT, tag="wkk")
            nc.sync.dma_start(out=wkk_sb, in_=wkk.rearrange("(c p) h m -> p c h m", p=128))
            wv_sb = per.tile([128, CT, 4 * D], AT_DT, tag="wv")
            nc.sync.dma_start(out=wv_sb, in_=wv.rearrange("(c p) m -> p c m", p=128))
            wproj_sb = per.tile([64, HPC, C], mm_dt, tag="wproj")
            nc.sync.dma_start(out=wproj_sb, in_=wproj.rearrange("(h p) m -> p h m", p=64))
            bias_sb = per.tile([1, C], mm_dt, tag="bias")
            nc.sync.dma_start(out=bias_sb, in_=bias)

            # --- persistent activations ---
            qt_sb = per.tile([128, HPC, N_Q], AT_DT, tag="qt")   # qT dup both halves
            kk_sb = per.tile([128, HPC, N_KV], AT_DT, tag="kk")  # kT dup both halves
            v1_sb = per.tile([128, HPC, N_KV // 128, D + 1], AT_DT, tag="v1")
            oTn_sb = per.tile([64, HPC, N_Q], mm_dt, tag="oTn")
            out_acc = per.tile([128, N_Q // 128, C], F32, tag="out_acc")

            # ================= phase 1: projections =====================
            with (
                tc.tile_pool(name="xTp", bufs=1) as xtp,
                tc.tile_pool(name="pstr", bufs=2, space="PSUM") as pstr,
                tc.tile_pool(name="pspr", bufs=2, space="PSUM") as pspr,
            ):
                qinT = xtp.tile([128, CT, N_Q], AT_DT, tag="qinT")
                nc.sync.dma_start(out=qinT,
                                  in_=q_inT.rearrange("(c p) n -> p c n", p=128))

                # qT projection: qT_h duplicated into both halves [128, n]
                for h in range(HPC):
                    for f in range(N_Q // 512):
                        pq = pspr.tile([128, 512], F32, tag="qp")
                        for c in range(CT):
                            nc.tensor.matmul(
                                pq, wq_sb[:, c, h, :],
                                qinT[:, c, 512 * f:512 * (f + 1)],
                                start=(c == 0), stop=(c == CT - 1))
                        nc.vector.tensor_copy(qt_sb[:, h, 512 * f:512 * (f + 1)], pq)

                # k projection: kT_h duplicated into both halves [128, N]
                xT_sb = xtp.tile([128, CT, N_KV], AT_DT, tag="xT")
                xT_r = xT.rearrange("(c p) n -> p c n", p=128)
                for c in range(CT):
                    nc.sync.dma_start(out=xT_sb[:, c, :], in_=xT_r[:, c, :])
                for h in range(HPC):
                    for f in range(N_KV // 512):
                        pk = pspr.tile([128, 512], F32, tag="kvp")
                        for c in range(CT):
                            nc.tensor.matmul(
                                pk, wkk_sb[:, c, h, :],
                                xT_sb[:, c, 512 * f:512 * (f + 1)],
                                start=(c == 0), stop=(c == CT - 1))
                        nc.vector.tensor_copy(kk_sb[:, h, 512 * f:512 * (f + 1)], pk)

                # v natural via direct projection: [N, 3*64] + ones column
                for g in range(N_KV // 128):
                    pv = pstr.tile([128, 4 * D], F32, tag="vp")
                    for c in range(CT):
                        nc.tensor.matmul(
                            pv, xT_sb[:, c, 128 * g:128 * (g + 1)],
                            wv_sb[:, c, :],
                            start=(c == 0), stop=(c == CT - 1))
                    nc.vector.tensor_copy(
                        v1_sb[:, :, g, 0:D],
                        pv[:, 0:HPC * D].rearrange("p (h d) -> p h d", h=HPC))
                for h in range(HPC):
                    nc.vector.tensor_copy(v1_sb[:, h, :, D], ones_f)

            # ============ phase 2: attention (A/B interleaved) ==========
            with (
                tc.tile_pool(name="ea", bufs=2) as eap,
                tc.tile_pool(name="et", bufs=4) as etp,
                tc.tile_pool(name="sm", bufs=2) as smp,
                tc.tile_pool(name="psA", bufs=2, space="PSUM") as psA,
                tc.tile_pool(name="psB", bufs=1, space="PSUM") as psB,
                tc.tile_pool(name="psO", bufs=1, space="PSUM") as psO,
            ):
                MT = N_KV // 128      # 32 pass-B m-tiles

                def emit_proj(h, last):
                    for i in range(N_Q // 128):
                        pp = psO.tile([128, C], F32, tag="po")
                        nc.tensor.matmul(pp[:, 0:512],
                                         oTn_sb[:, h, 128 * i:128 * (i + 1)],
                                         wproj_sb[:, h, 0:512],
                                         start=True, stop=(h != 0))
                        nc.tensor.matmul(pp[:, 512:768],
                                         oTn_sb[:, h, 128 * i:128 * (i + 1)],
                                         wproj_sb[:, h, 512:768],
                                         start=True, stop=(h != 0))
                        if h == 0:
                            nc.tensor.matmul(pp[:, 0:512], ones1, bias_sb[:, 0:512],
                                             start=False, stop=True)
                            nc.tensor.matmul(pp[:, 512:768], ones1,
                                             bias_sb[:, 512:768],
                                             start=False, stop=True)
                            nc.vector.tensor_copy(out_acc[:, i, :], pp)
                        else:
                            nc.vector.tensor_add(out_acc[:, i, :],
                                                 out_acc[:, i, :], pp)
                        if last:
                            nc.sync.dma_start(out=outp[128 * i:128 * (i + 1), :],
                                              in_=out_acc[:, i, :])

                def emit_proj_chunk(h, i, last):
                    pp = psB.tile([128, C], F32, tag="pb")
                    nc.tensor.matmul(pp[:, 0:512],
                                     oTn_sb[:, h, 128 * i:128 * (i + 1)],
                                     wproj_sb[:, h, 0:512],
                                     start=True, stop=(h != 0))
                    nc.tensor.matmul(pp[:, 512:768],
                                     oTn_sb[:, h, 128 * i:128 * (i + 1)],
                                     wproj_sb[:, h, 512:768],
                                     start=True, stop=(h != 0))
                    if h == 0:
                        nc.tensor.matmul(pp[:, 0:512], ones1, bias_sb[:, 0:512],
                                         start=False, stop=True)
                        nc.tensor.matmul(pp[:, 512:768], ones1,
                                         bias_sb[:, 512:768],
                                         start=False, stop=True)
                        nc.vector.tensor_copy(out_acc[:, i, :], pp)
                    else:
                        nc.vector.tensor_add(out_acc[:, i, :],
                                             out_acc[:, i, :], pp)
                    if last:
                        nc.sync.dma_start(out=outp[128 * i:128 * (i + 1), :],
                                          in_=out_acc[:, i, :])

                for h in range(HPC):
                    po = psO.tile([65, N_Q], F32, tag="po")
                    for i in range(N_Q // 128):
                        # previous head's projection, spread one chunk
                        # per iteration so the PE always has filler work
                        if h >= 1:
                            emit_proj_chunk(h - 1, i, last=False)
                        # ---- pass A tile i: scores [128, N] -> attn ----
                        ea = eap.tile([128, N_KV], F32, tag="ea")
                        zp = smp.tile([128, 4], F32, tag="zp")
                        for f in range(N_KV // 1024):
                            pa = psA.tile([128, 1024], F32, tag="pa")
                            for j in range(2):
                                lo, hi = (0, 64) if j == 0 else (64, 128)
                                nc.tensor.matmul(
                                    pa[:, 512 * j:512 * (j + 1)],
                                    qt_sb[lo:hi, h, 128 * i:128 * (i + 1)],
                                    kk_sb[lo:hi, h, 1024 * f + 512 * j:1024 * f + 512 * (j + 1)],
                                    start=True, stop=True)
                            nc.scalar.activation(
                                ea[:, 1024 * f:1024 * (f + 1)], pa, EXP_FUNC,
                                scale=SCALE, accum_out=zp[:, f:f + 1])
                        z = smp.tile([128, 1], F32, tag="z")
                        nc.vector.tensor_reduce(z, zp, axis=mybir.AxisListType.X,
                                                op=mybir.AluOpType.add)
                        rz = smp.tile([128, 1], F32, tag="rz")
                        nc.vector.reciprocal(rz, z)
                        nc.vector.tensor_scalar_mul(ea, ea, rz)
                        nc.sync.dma_start(
                            out=xattn[h, 128 * i:128 * (i + 1), :], in_=ea)

                        # ---- pass B m-tiles 4i..4i+3 ----
                        for m in range(4 * i, 4 * i + 4):
                            pb = psB.tile([128, N_Q], F32, tag="pb")
                            for j in range(2):
                                lo, hi = (0, 64) if j == 0 else (64, 128)
                                nc.tensor.matmul(
                                    pb[:, 512 * j:512 * (j + 1)],
                                    kk_sb[lo:hi, h, 128 * m:128 * (m + 1)],
                                    qt_sb[lo:hi, h, 512 * j:512 * (j + 1)],
                                    start=True, stop=True)
                            et = etp.tile([128, N_Q], AT_DT, tag="et")
                            nc.scalar.activation(et, pb, EXP_FUNC, scale=SCALE)
                            for j in range(2):
                                nc.tensor.matmul(
                                    po[:, 512 * j:512 * (j + 1)],
                                    v1_sb[:, h, m, :],
                                    et[:, 512 * j:512 * (j + 1)],
                                    start=(m == 0), stop=(m == MT - 1))

                    # ---- normalize oT by Z (row 64 of po) --------------
                    oT = smp.tile([65, N_Q], F32, tag="oT")
                    nc.vector.tensor_copy(oT, po)
                    zrow = smp.tile([1, N_Q], F32, tag="zrow")
                    nc.gpsimd.dma_start(out=zrow, in_=oT[64:65, :])
                    zrec = smp.tile([1, N_Q], F32, tag="zrec")
                    nc.vector.reciprocal(zrec, zrow)
                    bc = smp.tile([64, N_Q], F32, tag="bc")
                    nc.gpsimd.partition_broadcast(bc, zrec[0:1, :])
                    nc.vector.tensor_mul(oTn_sb[:, h, :], oT[0:64, :], bc)

                for i in range(N_Q // 128):
                    emit_proj_chunk(HPC - 1, i, last=True)

    nc.compile()
    return nc


_NC_CACHE = {}


def _get_nc():
    key = MM_DT
    if key not in _NC_CACHE:
        _NC_CACHE[key] = build(MM_DT)
    return _NC_CACHE[key]


def _make_in_maps(q_in, x, Wq, Wkv, Wproj, bproj):
    f32 = np.float32
    bf16 = ml_dtypes.bfloat16
    q_in = np.ascontiguousarray(q_in, dtype=f32)
    x = np.ascontiguousarray(x, dtype=f32)
    Wq = np.ascontiguousarray(Wq, dtype=f32)
    Wkv = np.ascontiguousarray(Wkv, dtype=f32)
    Wproj = np.ascontiguousarray(Wproj, dtype=f32)
    bproj = np.ascontiguousarray(bproj, dtype=f32)

    qTs = [np.ascontiguousarray(q_in[b].T).astype(bf16) for b in range(B)]
    xTs = [np.ascontiguousarray(x[b].T).astype(bf16) for b in range(B)]

    in_maps = []
    for core in range(8):
        b, hg = divmod(core, 4)
        heads = [HPC * hg + j for j in range(HPC)]
        wq_c = np.stack([np.concatenate([Wq[:, D * h:D * (h + 1)]] * 2, axis=1)
                         for h in heads], axis=1)              # [C, HPC, 2D] dup
        wkk_c = np.stack([np.concatenate([Wkv[:, D * h:D * (h + 1)]] * 2, axis=1)
                          for h in heads], axis=1)             # [C, HPC, 2D] dup
        wv_c = np.concatenate(
            [Wkv[:, C + D * h:C + D * (h + 1)] for h in heads]
            + [np.zeros((C, D), f32)], axis=1)                 # [C, 4D]
        wproj_c = np.concatenate([Wproj[D * h:D * (h + 1), :] for h in heads], axis=0)
        bias_c = bproj[None, :] if hg == 0 else np.zeros((1, C), f32)
        in_maps.append({
            "q_inT": qTs[b],
            "xT": xTs[b],
            "wq": np.ascontiguousarray(wq_c).astype(bf16),
            "wkk": np.ascontiguousarray(wkk_c).astype(bf16),
            "wv": np.ascontiguousarray(wv_c).astype(bf16),
            "wproj": np.ascontiguousarray(wproj_c),
            "bias": np.ascontiguousarray(bias_c),
        })
    return in_maps


def run(q_in, x, Wq, Wkv, Wproj, bproj, trace=False):
    nc = _get_nc()
    in_maps = _make_in_maps(q_in, x, Wq, Wkv, Wproj, bproj)
    res = bass_utils.run_bass_kernel_spmd(nc, in_maps, core_ids=list(range(8)),
                                          trace=trace)
    out = np.zeros((B, N_Q, C), np.float32)
    xattn = np.zeros((B, H, N_Q, N_KV), np.float32)
    for core in range(8):
        b, hg = divmod(core, 4)
        r = res.results[core]
        out[b] += r["out_part"]
        xattn[b, HPC * hg:HPC * (hg + 1)] = r["xattn_part"]
    return (out, xattn), res


def kernel(q_in, x, Wq, Wkv, Wproj, bproj):
    (out, xattn), _ = run(q_in, x, Wq, Wkv, Wproj, bproj)
    return out, xattn


# revision 18
# speedup vs baseline: 1.2841x; 1.2841x over previous
"""Cross-attention Trainium2 kernel (nn_CrossAttention_48842368090446).

Problem: B=2, n=1024, N=4096, C=768, H=12, D=64.
  q = (q_in @ Wq)  -> [B,H,n,D]
  k,v = (x @ Wkv)  -> [B,H,N,D] each
  xattn = softmax(q k^T / sqrt(D))    (returned, [B,H,n,N])
  out = (xattn @ v reshaped) @ Wproj + bproj    (returned, [B,n,C])

Sharding: 8 cores = 2 batches x 4 head-groups (3 heads each).
Each core gets its batch's q_inT/xT (host-transposed layout prep, cast
to bf16) plus its heads' weight slices, and produces xattn_part
[3,1024,4096] (fp32) and an out_part [1024,768] fp32 partial (summed
over the 4 cores of a batch on the host = the Wproj-row-sharded
all-reduce; bias counted once via the leader core's bias input).

On-core dataflow (per head, pass A/B interleaved to keep the PE warm):
  - qT=[D,n] and kvT=[kT|vT] packed [128,N] via bf16 projections (k on
    partitions 0:64, v on 64:128); v natural via PE transposes of vT.
  - pass A: scores [n,N] tiles on PE (bf16 in, fp32 PSUM) -> ACT
    exp(scale=1/8) with accum_out row sums -> DVE reciprocal +
    tensor_scalar normalize in place (fp32) -> DMA xattn.
  - pass B: scoresT [N,n] tiles -> ACT exp (bf16 out) -> PE
    attn@[v|ones] accumulated into oT[65,n] fp32 (row 64 = Z) ->
    gpsimd-broadcast 1/Z -> oT_norm (f32r).
  - proj (per head, f32r): partial = oT_norm_h^T @ Wproj_h (+ bias on
    head 0), accumulated into SBUF via DVE adds.
"""
import numpy as np
import ml_dtypes

import concourse.bacc as bacc
import concourse.mybir as mybir
import concourse.tile as tile
from concourse import bass_utils
from concourse.masks import make_identity

F32 = mybir.dt.float32
F32R = mybir.dt.float32r
BF16 = mybir.dt.bfloat16
MM_DT = F32R     # output projection: ~1.6e-4 rounding
AT_DT = BF16     # attention + q/kv projections

B, N_Q, N_KV, C, H, D = 2, 1024, 4096, 768, 12, 64
HPC = 3            # heads per core
CT = C // 128      # 6 contraction tiles
EXP_FUNC = mybir.ActivationFunctionType.Exp
SCALE = float(D) ** -0.5


def build(mm_dt=MM_DT):
    nc = bacc.Bacc("TRN2")

    q_inT = nc.dram_tensor("q_inT", [C, N_Q], AT_DT, kind="ExternalInput").ap()
    xT = nc.dram_tensor("xT", [C, N_KV], AT_DT, kind="ExternalInput").ap()
    wq = nc.dram_tensor("wq", [C, HPC, 2 * D], AT_DT, kind="ExternalInput").ap()
    wkk = nc.dram_tensor("wkk", [C, HPC, 2 * D], AT_DT, kind="ExternalInput").ap()
    wv = nc.dram_tensor("wv", [C, 4 * D], AT_DT, kind="ExternalInput").ap()
    wproj = nc.dram_tensor("wproj", [HPC * D, C], mm_dt, kind="ExternalInput").ap()
    bias = nc.dram_tensor("bias", [1, C], mm_dt, kind="ExternalInput").ap()
    xattn = nc.dram_tensor("xattn_part", [HPC, N_Q, N_KV], F32, kind="ExternalOutput").ap()
    outp = nc.dram_tensor("out_part", [N_Q, C], F32, kind="ExternalOutput").ap()

    with tile.TileContext(nc) as tc:
        with tc.tile_pool(name="per", bufs=1) as per:
            # --- constants / weights (persistent) ---
            ident_f = per.tile([128, 128], F32, tag="ident_f")
            make_identity(nc, ident_f)
            identb = per.tile([128, 128], AT_DT, tag="identb")
            nc.vector.tensor_copy(identb, ident_f)

            ones_f = per.tile([128, 32], F32, tag="ones_f")
            nc.vector.memset(ones_f, 1.0)
            ones1_f = per.tile([1, 128], F32, tag="ones1_f")
            nc.vector.memset(ones1_f, 1.0)
            ones1 = per.tile([1, 128], mm_dt, tag="ones1")
            nc.vector.tensor_copy(ones1, ones1_f)

            wq_sb = per.tile([128, CT, HPC, 2 * D], AT_DT, tag="wq")
            nc.sync.dma_start(out=wq_sb, in_=wq.rearrange("(c p) h m -> p c h m", p=128))
            wkk_sb = per.tile([128, CT, HPC, 2 * D], AT_DT, tag="wkk")
            nc.sync.dma_start(out=wkk_sb, in_=wkk.rearrange("(c p) h m -> p c h m", p=128))
            wv_sb = per.tile([128, CT, 4 * D], AT_DT, tag="wv")
            nc.sync.dma_start(out=wv_sb, in_=wv.rearrange("(c p) m -> p c m", p=128))
            wproj_sb = per.tile([64, HPC, C], mm_dt, tag="wproj")
            nc.sync.dma_start(out=wproj_sb, in_=wproj.rearrange("(h p) m -> p h m", p=64))
            bias_sb = per.tile([1, C], mm_dt, tag="bias")
            nc.sync.dma_start(out=bias_sb, in_=bias)

            # --- persistent activations ---
            qt_sb = per.tile([128, HPC, N_Q], AT_DT, tag="qt")   # qT dup both halves
            kk_sb = per.tile([128, HPC, N_KV], AT_DT, tag="kk")  # kT dup both halves
            v1_sb = per.tile([128, HPC, N_KV // 128, D + 1], AT_DT, tag="v1")
            oTn_sb = per.tile([64, HPC, N_Q], mm_dt, tag="oTn")
            out_acc = per.tile([128, N_Q // 128, C], F32, tag="out_acc")

            # ================= phase 1: projections =====================
            with (
                tc.tile_pool(name="xTp", bufs=1) as xtp,
                tc.tile_pool(name="pstr", bufs=2, space="PSUM") as pstr,
                tc.tile_pool(name="pspr", bufs=2, space="PSUM") as pspr,
            ):
                qinT = xtp.tile([128, CT, N_Q], AT_DT, tag="qinT")
                nc.sync.dma_start(out=qinT,
                                  in_=q_inT.rearrange("(c p) n -> p c n", p=128))

                # qT projection: qT_h duplicated into both halves [128, n]
                for h in range(HPC):
                    for f in range(N_Q // 512):
                        pq = pspr.tile([128, 512], F32, tag="qp")
                        for c in range(CT):
                            nc.tensor.matmul(
                                pq, wq_sb[:, c, h, :],
                                qinT[:, c, 512 * f:512 * (f + 1)],
                                start=(c == 0), stop=(c == CT - 1))
                        nc.vector.tensor_copy(qt_sb[:, h, 512 * f:512 * (f + 1)], pq)

                # k projection: kT_h duplicated into both halves [128, N]
                xT_sb = xtp.tile([128, CT, N_KV], AT_DT, tag="xT")
                xT_r = xT.rearrange("(c p) n -> p c n", p=128)
                for c in range(CT):
                    nc.sync.dma_start(out=xT_sb[:, c, :], in_=xT_r[:, c, :])
                for h in range(HPC):
                    for f in range(N_KV // 512):
                        pk = pspr.tile([128, 512], F32, tag="kvp")
                        for c in range(CT):
                            nc.tensor.matmul(
                                pk, wkk_sb[:, c, h, :],
                                xT_sb[:, c, 512 * f:512 * (f + 1)],
                                start=(c == 0), stop=(c == CT - 1))
                        nc.vector.tensor_copy(kk_sb[:, h, 512 * f:512 * (f + 1)], pk)

                # v natural via direct projection: [N, 3*64] + ones column
                for g in range(N_KV // 128):
                    pv = pstr.tile([128, 4 * D], F32, tag="vp")
                    for c in range(CT):
                        nc.tensor.matmul(
                            pv, xT_sb[:, c, 128 * g:128 * (g + 1)],
                            wv_sb[:, c, :],
                            start=(c == 0), stop=(c == CT - 1))
                    nc.vector.tensor_copy(
                        v1_sb[:, :, g, 0:D],
                        pv[:, 0:HPC * D].rearrange("p (h d) -> p h d", h=HPC))
                for h in range(HPC):
                    nc.vector.tensor_copy(v1_sb[:, h, :, D], ones_f)

            # ============ phase 2: attention (A/B interleaved) ==========
            with (
                tc.tile_pool(name="ea", bufs=2) as eap,
                tc.tile_pool(name="et", bufs=4) as etp,
                tc.tile_pool(name="sm", bufs=2) as smp,
                tc.tile_pool(name="psA", bufs=2, space="PSUM") as psA,
                tc.tile_pool(name="psB", bufs=1, space="PSUM") as psB,
                tc.tile_pool(name="psO", bufs=1, space="PSUM") as psO,
            ):
                MT = N_KV // 128      # 32 pass-B m-tiles

                def emit_proj(h, last):
                    for i in range(N_Q // 128):
                        pp = psO.tile([128, C], F32, tag="po")
                        nc.tensor.matmul(pp[:, 0:512],
                                         oTn_sb[:, h, 128 * i:128 * (i + 1)],
                                         wproj_sb[:, h, 0:512],
                                         start=True, stop=(h != 0))
                        nc.tensor.matmul(pp[:, 512:768],
                                         oTn_sb[:, h, 128 * i:128 * (i + 1)],
                                         wproj_sb[:, h, 512:768],
                                         start=True, stop=(h != 0))
                        if h == 0:
                            nc.tensor.matmul(pp[:, 0:512], ones1, bias_sb[:, 0:512],
                                             start=False, stop=True)
                            nc.tensor.matmul(pp[:, 512:768], ones1,
                                             bias_sb[:, 512:768],
                                             start=False, stop=True)
                            nc.vector.tensor_copy(out_acc[:, i, :], pp)
                        else:
                            nc.vector.tensor_add(out_acc[:, i, :],
                                                 out_acc[:, i, :], pp)
                        if last:
                            nc.sync.dma_start(out=outp[128 * i:128 * (i + 1), :],
                                              in_=out_acc[:, i, :])

                def emit_proj_chunk(h, i, last):
                    pp = psO.tile([128, C], F32, tag="po")
                    nc.tensor.matmul(pp[:, 0:512],
                                     oTn_sb[:, h, 128 * i:128 * (i + 1)],
                                     wproj_sb[:, h, 0:512],
                                     start=True, stop=(h != 0))
                    nc.tensor.matmul(pp[:, 512:768],
                                     oTn_sb[:, h, 128 * i:128 * (i + 1)],
                                     wproj_sb[:, h, 512:768],
                                     start=True, stop=(h != 0))
                    if h == 0:
                        nc.tensor.matmul(pp[:, 0:512], ones1, bias_sb[:, 0:512],
                                         start=False, stop=True)
                        nc.tensor.matmul(pp[:, 512:768], ones1,
                                         bias_sb[:, 512:768],
                                         start=False, stop=True)
                        nc.vector.tensor_copy(out_acc[:, i, :], pp)
                    else:
                        nc.vector.tensor_add(out_acc[:, i, :],
                                             out_acc[:, i, :], pp)
                    if last:
                        nc.sync.dma_start(out=outp[128 * i:128 * (i + 1), :],
                                          in_=out_acc[:, i, :])

                for h in range(HPC):
                    po = psO.tile([65, N_Q], F32, tag="po")
                    for i in range(N_Q // 128):
                        # previous head's projection, spread one chunk
                        # per iteration so the PE always has filler work
                        if h >= 1:
                            emit_proj_chunk(h - 1, i, last=False)
                        # ---- pass A tile i: scores [128, N] -> attn ----
                        ea = eap.tile([128, N_KV], F32, tag="ea")
                        zp = smp.tile([128, 4], F32, tag="zp")
                        for f in range(N_KV // 1024):
                            pa = psA.tile([128, 1024], F32, tag="pa")
                            for j in range(2):
                                lo, hi = (0, 64) if j == 0 else (64, 128)
                                nc.tensor.matmul(
                                    pa[:, 512 * j:512 * (j + 1)],
                                    qt_sb[lo:hi, h, 128 * i:128 * (i + 1)],
                                    kk_sb[lo:hi, h, 1024 * f + 512 * j:1024 * f + 512 * (j + 1)],
                                    start=True, stop=True)
                            nc.scalar.activation(
                                ea[:, 1024 * f:1024 * (f + 1)], pa, EXP_FUNC,
                                scale=SCALE, accum_out=zp[:, f:f + 1])
                        z = smp.tile([128, 1], F32, tag="z")
                        nc.vector.tensor_reduce(z, zp, axis=mybir.AxisListType.X,
                                                op=mybir.AluOpType.add)
                        rz = smp.tile([128, 1], F32, tag="rz")
                        nc.vector.reciprocal(rz, z)
                        nc.vector.tensor_scalar_mul(ea, ea, rz)
                        nc.sync.dma_start(
                            out=xattn[h, 128 * i:128 * (i + 1), :], in_=ea)

                        # ---- pass B m-tiles 4i..4i+3 ----
                        for m in range(4 * i, 4 * i + 4):
                            pb = psB.tile([128, N_Q], F32, tag="pb")
                            for j in range(2):
                                lo, hi = (0, 64) if j == 0 else (64, 128)
                                nc.tensor.matmul(
                                    pb[:, 512 * j:512 * (j + 1)],
                                    kk_sb[lo:hi, h, 128 * m:128 * (m + 1)],
                                    qt_sb[lo:hi, h, 512 * j:512 * (j + 1)],
                                    start=True, stop=True)
                            et = etp.tile([128, N_Q], AT_DT, tag="et")
                            nc.scalar.activation(et, pb, EXP_FUNC, scale=SCALE)
                            for j in range(2):
                                nc.tensor.matmul(
                                    po[:, 512 * j:512 * (j + 1)],
                                    v1_sb[:, h, m, :],
                                    et[:, 512 * j:512 * (j + 1)],
                                    start=(m == 0), stop=(m == MT - 1))

                    # ---- normalize oT by Z (row 64 of po) --------------
                    oT = smp.tile([65, N_Q], F32, tag="oT")
                    nc.vector.tensor_copy(oT, po)
                    zrow = smp.tile([1, N_Q], F32, tag="zrow")
                    nc.gpsimd.dma_start(out=zrow, in_=oT[64:65, :])
                    zrec = smp.tile([1, N_Q], F32, tag="zrec")
                    nc.vector.reciprocal(zrec, zrow)
                    bc = smp.tile([64, N_Q], F32, tag="bc")
                    nc.gpsimd.partition_broadcast(bc, zrec[0:1, :])
                    nc.vector.tensor_mul(oTn_sb[:, h, :], oT[0:64, :], bc)

                for i in range(N_Q // 128):
                    emit_proj_chunk(HPC - 1, i, last=True)

    nc.compile()
    return nc


_NC_CACHE = {}


def _get_nc():
    key = MM_DT
    if key not in _NC_CACHE:
        _NC_CACHE[key] = build(MM_DT)
    return _NC_CACHE[key]


def _make_in_maps(q_in, x, Wq, Wkv, Wproj, bproj):
    f32 = np.float32
    bf16 = ml_dtypes.bfloat16
    q_in = np.ascontiguousarray(q_in, dtype=f32)
    x = np.ascontiguousarray(x, dtype=f32)
    Wq = np.ascontiguousarray(Wq, dtype=f32)
    Wkv = np.ascontiguousarray(Wkv, dtype=f32)
    Wproj = np.ascontiguousarray(Wproj, dtype=f32)
    bproj = np.ascontiguousarray(bproj, dtype=f32)

    qTs = [np.ascontiguousarray(q_in[b].T).astype(bf16) for b in range(B)]
    xTs = [np.ascontiguousarray(x[b].T).astype(bf16) for b in range(B)]

    in_maps = []
    for core in range(8):
        b, hg = divmod(core, 4)
        heads = [HPC * hg + j for j in range(HPC)]
        wq_c = np.stack([np.concatenate([Wq[:, D * h:D * (h + 1)]] * 2, axis=1)
                         for h in heads], axis=1)              # [C, HPC, 2D] dup
        wkk_c = np.stack([np.concatenate([Wkv[:, D * h:D * (h + 1)]] * 2, axis=1)
                          for h in heads], axis=1)             # [C, HPC, 2D] dup
        wv_c = np.concatenate(
            [Wkv[:, C + D * h:C + D * (h + 1)] for h in heads]
            + [np.zeros((C, D), f32)], axis=1)                 # [C, 4D]
        wproj_c = np.concatenate([Wproj[D * h:D * (h + 1), :] for h in heads], axis=0)
        bias_c = bproj[None, :] if hg == 0 else np.zeros((1, C), f32)
        in_maps.append({
            "q_inT": qTs[b],
            "xT": xTs[b],
            "wq": np.ascontiguousarray(wq_c).astype(bf16),
            "wkk": np.ascontiguousarray(wkk_c).astype(bf16),
            "wv": np.ascontiguousarray(wv_c).astype(bf16),
            "wproj": np.ascontiguousarray(wproj_c),
            "bias": np.ascontiguousarray(bias_c),
        })
    return in_maps


def run(q_in, x, Wq, Wkv, Wproj, bproj, trace=False):
    nc = _get_nc()
    in_maps = _make_in_maps(q_in, x, Wq, Wkv, Wproj, bproj)
    res = bass_utils.run_bass_kernel_spmd(nc, in_maps, core_ids=list(range(8)),
                                          trace=trace)
    out = np.zeros((B, N_Q, C), np.float32)
    xattn = np.zeros((B, H, N_Q, N_KV), np.float32)
    for core in range(8):
        b, hg = divmod(core, 4)
        r = res.results[core]
        out[b] += r["out_part"]
        xattn[b, HPC * hg:HPC * (hg + 1)] = r["xattn_part"]
    return (out, xattn), res


def kernel(q_in, x, Wq, Wkv, Wproj, bproj):
    (out, xattn), _ = run(q_in, x, Wq, Wkv, Wproj, bproj)
    return out, xattn


# revision 19
# speedup vs baseline: 1.4690x; 1.1440x over previous
"""Cross-attention Trainium2 kernel (nn_CrossAttention_48842368090446).

Problem: B=2, n=1024, N=4096, C=768, H=12, D=64.
  q = (q_in @ Wq)  -> [B,H,n,D]
  k,v = (x @ Wkv)  -> [B,H,N,D] each
  xattn = softmax(q k^T / sqrt(D))    (returned, [B,H,n,N])
  out = (xattn @ v reshaped) @ Wproj + bproj    (returned, [B,n,C])

Sharding: 8 cores = 2 batches x 4 head-groups (3 heads each).
Each core gets its batch's q_inT/xT (host-transposed layout prep, cast
to bf16) plus its heads' weight slices, and produces xattn_part
[3,1024,4096] (fp32) and an out_part [1024,768] fp32 partial (summed
over the 4 cores of a batch on the host = the Wproj-row-sharded
all-reduce; bias counted once via the leader core's bias input).

On-core dataflow (per head, pass A/B interleaved to keep the PE warm):
  - qT=[D,n] and kvT=[kT|vT] packed [128,N] via bf16 projections (k on
    partitions 0:64, v on 64:128); v natural via PE transposes of vT.
  - pass A: scores [n,N] tiles on PE (bf16 in, fp32 PSUM) -> ACT
    exp(scale=1/8) with accum_out row sums -> DVE reciprocal +
    tensor_scalar normalize in place (fp32) -> DMA xattn.
  - pass B: scoresT [N,n] tiles -> ACT exp (bf16 out) -> PE
    attn@[v|ones] accumulated into oT[65,n] fp32 (row 64 = Z) ->
    gpsimd-broadcast 1/Z -> oT_norm (f32r).
  - proj (per head, f32r): partial = oT_norm_h^T @ Wproj_h (+ bias on
    head 0), accumulated into SBUF via DVE adds.
"""
import numpy as np
import ml_dtypes

import concourse.bacc as bacc
import concourse.mybir as mybir
import concourse.tile as tile
from concourse import bass_utils
from concourse.masks import make_identity

F32 = mybir.dt.float32
F32R = mybir.dt.float32r
BF16 = mybir.dt.bfloat16
MM_DT = F32R     # output projection: ~1.6e-4 rounding
AT_DT = BF16     # attention + q/kv projections

B, N_Q, N_KV, C, H, D = 2, 1024, 4096, 768, 12, 64
HPC = 3            # heads per core
CT = C // 128      # 6 contraction tiles
EXP_FUNC = mybir.ActivationFunctionType.Exp
SCALE = float(D) ** -0.5


def build(mm_dt=MM_DT):
    nc = bacc.Bacc("TRN2")

    q_inT = nc.dram_tensor("q_inT", [C, N_Q], AT_DT, kind="ExternalInput").ap()
    xT = nc.dram_tensor("xT", [C, N_KV], AT_DT, kind="ExternalInput").ap()
    wq = nc.dram_tensor("wq", [C, HPC, 2 * D], AT_DT, kind="ExternalInput").ap()
    wkk = nc.dram_tensor("wkk", [C, HPC, 2 * D], AT_DT, kind="ExternalInput").ap()
    wv = nc.dram_tensor("wv", [C, 4 * D], AT_DT, kind="ExternalInput").ap()
    wproj = nc.dram_tensor("wproj", [HPC * D, C], mm_dt, kind="ExternalInput").ap()
    bias = nc.dram_tensor("bias", [1, C], mm_dt, kind="ExternalInput").ap()
    xattn = nc.dram_tensor("xattn_part", [HPC, N_Q, N_KV], F32, kind="ExternalOutput").ap()
    outp = nc.dram_tensor("out_part", [N_Q, C], F32, kind="ExternalOutput").ap()

    with tile.TileContext(nc) as tc:
        with tc.tile_pool(name="per", bufs=1) as per:
            # --- constants / weights (persistent) ---
            ident_f = per.tile([128, 128], F32, tag="ident_f")
            make_identity(nc, ident_f)
            identb = per.tile([128, 128], AT_DT, tag="identb")
            nc.vector.tensor_copy(identb, ident_f)

            ones_f = per.tile([128, 32], F32, tag="ones_f")
            nc.vector.memset(ones_f, 1.0)
            ones1_f = per.tile([1, 128], F32, tag="ones1_f")
            nc.vector.memset(ones1_f, 1.0)
            ones1 = per.tile([1, 128], mm_dt, tag="ones1")
            nc.vector.tensor_copy(ones1, ones1_f)

            wq_sb = per.tile([128, CT, HPC, 2 * D], AT_DT, tag="wq")
            nc.sync.dma_start(out=wq_sb, in_=wq.rearrange("(c p) h m -> p c h m", p=128))
            wkk_sb = per.tile([128, CT, HPC, 2 * D], AT_DT, tag="wkk")
            nc.sync.dma_start(out=wkk_sb, in_=wkk.rearrange("(c p) h m -> p c h m", p=128))
            wv_sb = per.tile([128, CT, 4 * D], AT_DT, tag="wv")
            nc.sync.dma_start(out=wv_sb, in_=wv.rearrange("(c p) m -> p c m", p=128))
            wproj_sb = per.tile([64, HPC, C], mm_dt, tag="wproj")
            nc.sync.dma_start(out=wproj_sb, in_=wproj.rearrange("(h p) m -> p h m", p=64))
            bias_sb = per.tile([1, C], mm_dt, tag="bias")
            nc.sync.dma_start(out=bias_sb, in_=bias)

            # --- persistent activations ---
            qt_sb = per.tile([128, HPC, N_Q], AT_DT, tag="qt")   # qT dup both halves
            kk_sb = per.tile([128, HPC, N_KV], AT_DT, tag="kk")  # kT dup both halves
            v1_sb = per.tile([128, HPC, N_KV // 128, D + 1], AT_DT, tag="v1")
            oTn_sb = per.tile([64, HPC, N_Q], mm_dt, tag="oTn")
            out_acc = per.tile([128, N_Q // 128, C], F32, tag="out_acc")

            # ================= phase 1: projections =====================
            with (
                tc.tile_pool(name="xTp", bufs=1) as xtp,
                tc.tile_pool(name="pstr", bufs=2, space="PSUM") as pstr,
                tc.tile_pool(name="pspr", bufs=2, space="PSUM") as pspr,
            ):
                qinT = xtp.tile([128, CT, N_Q], AT_DT, tag="qinT")
                nc.sync.dma_start(out=qinT,
                                  in_=q_inT.rearrange("(c p) n -> p c n", p=128))

                # qT projection: qT_h duplicated into both halves [128, n]
                for h in range(HPC):
                    for f in range(N_Q // 512):
                        pq = pspr.tile([128, 512], F32, tag="qp")
                        for c in range(CT):
                            nc.tensor.matmul(
                                pq, wq_sb[:, c, h, :],
                                qinT[:, c, 512 * f:512 * (f + 1)],
                                start=(c == 0), stop=(c == CT - 1))
                        nc.vector.tensor_copy(qt_sb[:, h, 512 * f:512 * (f + 1)], pq)

                # k projection: kT_h duplicated into both halves [128, N]
                xT_sb = xtp.tile([128, CT, N_KV], AT_DT, tag="xT")
                xT_r = xT.rearrange("(c p) n -> p c n", p=128)
                for c in range(CT):
                    nc.sync.dma_start(out=xT_sb[:, c, :], in_=xT_r[:, c, :])
                for h in range(HPC):
                    for f in range(N_KV // 512):
                        pk = pspr.tile([128, 512], F32, tag="kvp")
                        for c in range(CT):
                            nc.tensor.matmul(
                                pk, wkk_sb[:, c, h, :],
                                xT_sb[:, c, 512 * f:512 * (f + 1)],
                                start=(c == 0), stop=(c == CT - 1))
                        nc.vector.tensor_copy(kk_sb[:, h, 512 * f:512 * (f + 1)], pk)

                # v natural via direct projection: [N, 3*64] + ones column
                for g in range(N_KV // 128):
                    pv = pstr.tile([128, 4 * D], F32, tag="vp")
                    for c in range(CT):
                        nc.tensor.matmul(
                            pv, xT_sb[:, c, 128 * g:128 * (g + 1)],
                            wv_sb[:, c, :],
                            start=(c == 0), stop=(c == CT - 1))
                    nc.vector.tensor_copy(
                        v1_sb[:, :, g, 0:D],
                        pv[:, 0:HPC * D].rearrange("p (h d) -> p h d", h=HPC))
                for h in range(HPC):
                    nc.vector.tensor_copy(v1_sb[:, h, :, D], ones_f)

            # ============ phase 2: attention (A/B interleaved) ==========
            with (
                tc.tile_pool(name="ea", bufs=2) as eap,
                tc.tile_pool(name="et", bufs=4) as etp,
                tc.tile_pool(name="sm", bufs=2) as smp,
                tc.tile_pool(name="psA", bufs=1, space="PSUM") as psA,
                tc.tile_pool(name="psB", bufs=2, space="PSUM") as psB,
                tc.tile_pool(name="psO", bufs=1, space="PSUM") as psO,
            ):
                MT = N_KV // 128      # 32 pass-B m-tiles

                def emit_proj(h, last):
                    for i in range(N_Q // 128):
                        pp = psO.tile([128, C], F32, tag="po")
                        nc.tensor.matmul(pp[:, 0:512],
                                         oTn_sb[:, h, 128 * i:128 * (i + 1)],
                                         wproj_sb[:, h, 0:512],
                                         start=True, stop=(h != 0))
                        nc.tensor.matmul(pp[:, 512:768],
                                         oTn_sb[:, h, 128 * i:128 * (i + 1)],
                                         wproj_sb[:, h, 512:768],
                                         start=True, stop=(h != 0))
                        if h == 0:
                            nc.tensor.matmul(pp[:, 0:512], ones1, bias_sb[:, 0:512],
                                             start=False, stop=True)
                            nc.tensor.matmul(pp[:, 512:768], ones1,
                                             bias_sb[:, 512:768],
                                             start=False, stop=True)
                            nc.vector.tensor_copy(out_acc[:, i, :], pp)
                        else:
                            nc.vector.tensor_add(out_acc[:, i, :],
                                                 out_acc[:, i, :], pp)
                        if last:
                            nc.sync.dma_start(out=outp[128 * i:128 * (i + 1), :],
                                              in_=out_acc[:, i, :])

                def emit_proj_chunk(h, i, last):
                    pp = psO.tile([128, C], F32, tag="po")
                    nc.tensor.matmul(pp[:, 0:512],
                                     oTn_sb[:, h, 128 * i:128 * (i + 1)],
                                     wproj_sb[:, h, 0:512],
                                     start=True, stop=(h != 0))
                    nc.tensor.matmul(pp[:, 512:768],
                                     oTn_sb[:, h, 128 * i:128 * (i + 1)],
                                     wproj_sb[:, h, 512:768],
                                     start=True, stop=(h != 0))
                    if h == 0:
                        nc.tensor.matmul(pp[:, 0:512], ones1, bias_sb[:, 0:512],
                                         start=False, stop=True)
                        nc.tensor.matmul(pp[:, 512:768], ones1,
                                         bias_sb[:, 512:768],
                                         start=False, stop=True)
                        nc.vector.tensor_copy(out_acc[:, i, :], pp)
                    else:
                        nc.vector.tensor_add(out_acc[:, i, :],
                                             out_acc[:, i, :], pp)
                    if last:
                        nc.sync.dma_start(out=outp[128 * i:128 * (i + 1), :],
                                          in_=out_acc[:, i, :])

                for h in range(HPC):
                    po = psO.tile([65, N_Q], F32, tag="po")
                    for i in range(N_Q // 128):
                        # previous head's projection, spread one chunk
                        # per iteration so the PE always has filler work
                        if h >= 1:
                            emit_proj_chunk(h - 1, i, last=False)
                        # ---- pass A tile i: scores [128, N] -> attn ----
                        ea = eap.tile([128, N_KV], F32, tag="ea")
                        zp = smp.tile([128, 4], F32, tag="zp")
                        for f in range(N_KV // 1024):
                            pa = psA.tile([128, 1024], F32, tag="pa")
                            for j in range(2):
                                lo, hi = (0, 64) if j == 0 else (64, 128)
                                nc.tensor.matmul(
                                    pa[:, 512 * j:512 * (j + 1)],
                                    qt_sb[lo:hi, h, 128 * i:128 * (i + 1)],
                                    kk_sb[lo:hi, h, 1024 * f + 512 * j:1024 * f + 512 * (j + 1)],
                                    start=True, stop=True)
                            nc.scalar.activation(
                                ea[:, 1024 * f:1024 * (f + 1)], pa, EXP_FUNC,
                                scale=SCALE, accum_out=zp[:, f:f + 1])
                        z = smp.tile([128, 1], F32, tag="z")
                        nc.vector.tensor_reduce(z, zp, axis=mybir.AxisListType.X,
                                                op=mybir.AluOpType.add)
                        rz = smp.tile([128, 1], F32, tag="rz")
                        nc.vector.reciprocal(rz, z)
                        nc.vector.tensor_scalar_mul(ea, ea, rz)
                        nc.sync.dma_start(
                            out=xattn[h, 128 * i:128 * (i + 1), :], in_=ea)

                        # ---- pass B m-tiles 4i..4i+3 ----
                        for m in range(4 * i, 4 * i + 4):
                            pb = psB.tile([128, N_Q], F32, tag="pb")
                            for j in range(2):
                                lo, hi = (0, 64) if j == 0 else (64, 128)
                                nc.tensor.matmul(
                                    pb[:, 512 * j:512 * (j + 1)],
                                    kk_sb[lo:hi, h, 128 * m:128 * (m + 1)],
                                    qt_sb[lo:hi, h, 512 * j:512 * (j + 1)],
                                    start=True, stop=True)
                            et = etp.tile([128, N_Q], AT_DT, tag="et")
                            nc.scalar.activation(et, pb, EXP_FUNC, scale=SCALE)
                            for j in range(2):
                                nc.tensor.matmul(
                                    po[:, 512 * j:512 * (j + 1)],
                                    v1_sb[:, h, m, :],
                                    et[:, 512 * j:512 * (j + 1)],
                                    start=(m == 0), stop=(m == MT - 1))

                    # ---- normalize oT by Z (row 64 of po) --------------
                    oT = smp.tile([65, N_Q], F32, tag="oT")
                    nc.vector.tensor_copy(oT, po)
                    zrow = smp.tile([1, N_Q], F32, tag="zrow")
                    nc.gpsimd.dma_start(out=zrow, in_=oT[64:65, :])
                    zrec = smp.tile([1, N_Q], F32, tag="zrec")
                    nc.vector.reciprocal(zrec, zrow)
                    bc = smp.tile([64, N_Q], F32, tag="bc")
                    nc.gpsimd.partition_broadcast(bc, zrec[0:1, :])
                    nc.vector.tensor_mul(oTn_sb[:, h, :], oT[0:64, :], bc)

                for i in range(N_Q // 128):
                    emit_proj_chunk(HPC - 1, i, last=True)

    nc.compile()
    return nc


_NC_CACHE = {}


def _get_nc():
    key = MM_DT
    if key not in _NC_CACHE:
        _NC_CACHE[key] = build(MM_DT)
    return _NC_CACHE[key]


def _make_in_maps(q_in, x, Wq, Wkv, Wproj, bproj):
    f32 = np.float32
    bf16 = ml_dtypes.bfloat16
    q_in = np.ascontiguousarray(q_in, dtype=f32)
    x = np.ascontiguousarray(x, dtype=f32)
    Wq = np.ascontiguousarray(Wq, dtype=f32)
    Wkv = np.ascontiguousarray(Wkv, dtype=f32)
    Wproj = np.ascontiguousarray(Wproj, dtype=f32)
    bproj = np.ascontiguousarray(bproj, dtype=f32)

    qTs = [np.ascontiguousarray(q_in[b].T).astype(bf16) for b in range(B)]
    xTs = [np.ascontiguousarray(x[b].T).astype(bf16) for b in range(B)]

    in_maps = []
    for core in range(8):
        b, hg = divmod(core, 4)
        heads = [HPC * hg + j for j in range(HPC)]
        wq_c = np.stack([np.concatenate([Wq[:, D * h:D * (h + 1)]] * 2, axis=1)
                         for h in heads], axis=1)              # [C, HPC, 2D] dup
        wkk_c = np.stack([np.concatenate([Wkv[:, D * h:D * (h + 1)]] * 2, axis=1)
                          for h in heads], axis=1)             # [C, HPC, 2D] dup
        wv_c = np.concatenate(
            [Wkv[:, C + D * h:C + D * (h + 1)] for h in heads]
            + [np.zeros((C, D), f32)], axis=1)                 # [C, 4D]
        wproj_c = np.concatenate([Wproj[D * h:D * (h + 1), :] for h in heads], axis=0)
        bias_c = bproj[None, :] if hg == 0 else np.zeros((1, C), f32)
        in_maps.append({
            "q_inT": qTs[b],
            "xT": xTs[b],
            "wq": np.ascontiguousarray(wq_c).astype(bf16),
            "wkk": np.ascontiguousarray(wkk_c).astype(bf16),
            "wv": np.ascontiguousarray(wv_c).astype(bf16),
            "wproj": np.ascontiguousarray(wproj_c),
            "bias": np.ascontiguousarray(bias_c),
        })
    return in_maps


def run(q_in, x, Wq, Wkv, Wproj, bproj, trace=False):
    nc = _get_nc()
    in_maps = _make_in_maps(q_in, x, Wq, Wkv, Wproj, bproj)
    res = bass_utils.run_bass_kernel_spmd(nc, in_maps, core_ids=list(range(8)),
                                          trace=trace)
    out = np.zeros((B, N_Q, C), np.float32)
    xattn = np.zeros((B, H, N_Q, N_KV), np.float32)
    for core in range(8):
        b, hg = divmod(core, 4)
        r = res.results[core]
        out[b] += r["out_part"]
        xattn[b, HPC * hg:HPC * (hg + 1)] = r["xattn_part"]
    return (out, xattn), res


def kernel(q_in, x, Wq, Wkv, Wproj, bproj):
    (out, xattn), _ = run(q_in, x, Wq, Wkv, Wproj, bproj)
    return out, xattn
